# revision 50
# baseline (speedup 1.0000x reference)
"""GAT-style GNN message passing on 8 TRN2 NeuronCores.

Math: with LEAK=1 the leaky-relu is identity, so softmax over j cancels
e_src exactly:
  out[i,(h,f)] = (adj @ (z*h))[i,(h,f)] / (adj @ z)[i,h],  z = exp(h @ a_dst)
then elu + log_softmax per row (elu(x)+1 = relu(x)+exp(min(x,0)); the +1
is a uniform shift which log_softmax cancels).

Parallelisation: rows (query nodes) of adj/out are sharded across the 8
cores, but x is REPLICATED and every core computes the full [N, 72]
G = [h*z | z] locally. The AllGather alternative (gather 40KB of G per
core) measures ~46us end-to-end on this runtime (ncfw trigger->mesh
latency dominates) and couples the cores' launch skew; replicating the
x read costs only ~10us of extra DMA and makes each core's execution
completely independent.

All matmuls run as fp8e4 DoubleRow (2 contractions/cycle):
 - adj entries are 0/1 (exact in fp8); per-core slab is 2MB.
 - x, W are fp8 with power-of-2 column scaling so values sit in e4m3's
   normal range: h' = x @ (16W) = 16h, e' = x @ (64Wd) = 64*e_dst.
   z = exp(e'/64) (ACT scale), g = (h'/16)*z via one scalar_tensor_tensor
   that reuses the rounded fp8 z (numerator and denominator then share
   the same z, so z rounding only reweights attention by ~0.06/sqrt(2048)).
 - dominant error is g/h rounding ~10% of the signal; |out| ~ 0.04 vs
   log_softmax scale ~4.6 => rel err ~ 2e-3, well under the 2e-2 gate.
 - E is padded 72->80 so the DoubleRow pair stride (80B) is 16B-aligned
   (walrus double_row_stride_alignment). Pad columns only produce junk
   PSUM rows 72:80 which are never read (zeroed once for determinism).

The 2MB adjacency streams behind a real data dependency on the x load
(Tile hoists dependency-free DMAs, so the gate is one garbage byte
copied from xt_sb into each chunk's region, overwritten by the DMA);
it is only needed by the aggregation ~10us after x completes.

Per-core layouts (R=512 out-rows/core, P=128, KK=4 k-blocks of 256,
G=8 node column-groups of 512):
  xt [128, 8*4*2*512] fp8  xt[p, g,kk,s,r] = x[g*512 + r, kk*256+s*128+p]
                           (full x, same buffer on every core)
  wt [128, 4*2*80]    fp8  wt[p, kk,s,e]   = w_ext[kk*256+s*128+p, e]
  at [128, 16*2*512]  fp8  at[p, blk,s,r]  = adj[c*R + r, blk*256+s*128+p]
  out [128, 4*64]     f32  out[p, q*64+f]  = out[c*R + q*128 + p, f]
"""

import sys

import numpy as np

if "/opt/trn_rl_repo" not in sys.path:
    sys.path.insert(0, "/opt/trn_rl_repo")

import ml_dtypes  # noqa: E402

import concourse.bass as bass  # noqa: E402
import concourse.tile as tile  # noqa: E402
from concourse import bacc, mybir  # noqa: E402
from concourse.bass_utils import run_bass_kernel_spmd  # noqa: E402
from concourse.masks import make_identity  # noqa: E402

N_CORES = 8
H = 8
F = 8
HF = H * F  # 64
EXT = HF + H  # 72: [g | z]
EP = 80  # padded to 16B-aligned DoubleRow pair stride
K_IN = 1024
P = 128
KK = K_IN // 256  # 4 double-row k-blocks for the h matmul

FP32 = mybir.dt.float32
FP8 = mybir.dt.float8e4
AFT = mybir.ActivationFunctionType
ALU = mybir.AluOpType
DR = mybir.MatmulPerfMode.DoubleRow


def _bcast_f(ap_ph, n):
    """[P, H] AP -> [P, H, n] AP broadcasting each head value over n."""
    return bass.AP(
        tensor=ap_ph.tensor,
        offset=ap_ph.offset,
        ap=[ap_ph.ap[0], ap_ph.ap[1], [0, n]],
    )


def build_bass(n_nodes: int) -> bass.Bass:
    R = n_nodes // N_CORES  # 512 output rows per core
    NG = n_nodes // R  # 8 node column-groups (of 512) for the h matmul
    NBLK = n_nodes // 256  # 16 double-row j-blocks for the aggregation
    RC = R // P  # 4 output row chunks per core

    nc = bacc.Bacc(num_devices=N_CORES)

    xt = nc.declare_dram_parameter(
        "xt", [P, NG * KK * 2 * (R // 1)], FP8, isOutput=False
    )
    at = nc.declare_dram_parameter("at", [P, NBLK * 2 * R], FP8, isOutput=False)
    wt = nc.declare_dram_parameter("wt", [P, KK * 2 * EP], FP8, isOutput=False)
    out = nc.declare_dram_parameter("out", [P, RC * HF], FP32, isOutput=True)

    with tile.TileContext(nc) as tc:
        with (
            tc.tile_pool(name="singles", bufs=1) as singles,
            tc.tile_pool(name="bigpsum", bufs=2, space="PSUM") as bigpsum,
            tc.tile_pool(name="smallpsum", bufs=4, space="PSUM") as smallpsum,
            tc.tile_pool(name="opsum", bufs=1, space="PSUM") as opsum,
            tc.tile_pool(name="aggpsum", bufs=1, space="PSUM") as aggpsum,
            tc.tile_pool(name="work", bufs=4) as work,
        ):
            ident = singles.tile([P, P], FP32)
            make_identity(nc, ident)
            identb = singles.tile([P, P], mybir.dt.bfloat16)
            make_identity(nc, identb)

            # --- critical-path loads: wt first, then x group by group ---
            w_sb = singles.tile([P, KK, 2, EP], FP8)
            nc.sync.dma_start(
                out=w_sb, in_=wt[:].rearrange("p (k s e) -> p k s e", k=KK, s=2)
            )
            xt_sb = singles.tile([P, NG, KK, 2, R], FP8)
            xt_view = xt[:].rearrange(
                "p (g k s r) -> p g k s r", g=NG, k=KK, s=2
            )
            # first two groups load alone so the PE starts ~2us earlier
            xt_groups = [(0, 1), (1, 2), (2, 4), (4, 6), (6, 8)]
            for lo, hi in xt_groups:
                nc.sync.dma_start(out=xt_sb[:, lo:hi], in_=xt_view[:, lo:hi])

            # --- adjacency gated behind the x load (SWDGE ring) ---
            at_sb = singles.tile([P, NBLK, 2, R], FP8)
            at_view = at[:].rearrange("p (b s r) -> p b s r", b=NBLK, s=2)
            N_SPLITS = 4
            # gate on xt group 2: early enough that the adjacency's last
            # chunk lands well before the aggregation (a later gate stalls
            # the agg matmuls: measured +6us at group 6), at the cost of
            # some bandwidth-sharing with the xt tail
            for sp in range(N_SPLITS):
                lo = NBLK // N_SPLITS * sp
                nc.vector.tensor_copy(
                    at_sb[0:1, lo, 0, 0:1],
                    xt_sb[0:1, 2, 0, 0, sp : sp + 1],
                )
            for sp in range(N_SPLITS):
                lo, hi = NBLK // N_SPLITS * sp, NBLK // N_SPLITS * (sp + 1)
                nc.gpsimd.dma_start(out=at_sb[:, lo:hi], in_=at_view[:, lo:hi])

            # --- full G production: per column-group of 512 nodes,
            # h' = (16W|64Wd).T @ x.T, transpose 128-chunks,
            # z = exp(e'/64), g = (h'/16)*z ---
            g2 = singles.tile([P, NBLK, 2, EP], FP8)
            g2_base = g2[:, 0, 0, :]
            pad_ap = bass.AP(
                tensor=g2_base.tensor,
                offset=g2_base.offset + EXT,
                ap=[g2_base.ap[0], [EP, NBLK * 2], [1, EP - EXT]],
            )
            nc.vector.memset(pad_ap, 0.0)

            # bf16 staging: halves the PSUM->SBUF copy and the transpose
            # traffic; h'/e' at 0.4% rel err is negligible vs fp8's 6%
            BF16 = mybir.dt.bfloat16
            hT_sb = singles.tile([EXT, NG, R], BF16)
            outT_ps = aggpsum.tile([EP, R], FP32, tag="aggps")
            for g in range(NG):
                hT_ps = bigpsum.tile([EP, R], FP32, tag="bigps", name=f"hT{g}")
                for k in range(KK):
                    nc.tensor.matmul(
                        hT_ps,
                        lhsT=w_sb[:, k],
                        rhs=xt_sb[:, g, k],
                        start=(k == 0),
                        stop=(k == KK - 1),
                        perf_mode=DR,
                    )
                # staging copy on ACT (reads PSUM fast, otherwise idle here);
                # frees DVE for the scalar_tensor_tensor ladder
                nc.scalar.activation(hT_sb[:, g], hT_ps[:EXT], AFT.Copy)
                for qq in range(RC):
                    q = g * RC + qq
                    h_ps = smallpsum.tile([P, EXT], mybir.dt.bfloat16, tag="smallps")
                    nc.tensor.transpose(
                        h_ps,
                        hT_sb[:, g, qq * P : (qq + 1) * P],
                        identb[:EXT, :EXT],
                    )
                    zslice = g2[:, q // 2, q % 2, HF:EXT]
                    nc.scalar.activation(
                        zslice, h_ps[:, HF:EXT], AFT.Exp, scale=1.0 / 64.0
                    )
                    nc.vector.scalar_tensor_tensor(
                        out=g2[:, q // 2, q % 2, 0:HF].rearrange(
                            "p (h f) -> p h f", h=H
                        ),
                        in0=h_ps[:, 0:HF].rearrange("p (h f) -> p h f", h=H),
                        scalar=1.0 / 16.0,
                        in1=_bcast_f(zslice, F),
                        op0=ALU.mult,
                        op1=ALU.mult,
                    )
            # --- aggregation: outT += G_blk.T @ adjT_blk (16 DR matmuls;
            # kept after the group loop — interleaving them stalls the FIFO
            # PE queue on adjacency chunks that are still streaming) ---
            for blk in range(NBLK):
                nc.tensor.matmul(
                    outT_ps,
                    lhsT=g2[:, blk],
                    rhs=at_sb[:, blk],
                    start=(blk == 0),
                    stop=(blk == NBLK - 1),
                    perf_mode=DR,
                )
            outT_sb = singles.tile([EXT, R], FP32)
            nc.vector.tensor_copy(outT_sb, outT_ps[:EXT])

            # --- postprocess: all 4 transposed chunks land in ONE PSUM bank
            # (start=True only clears has_written bits, data in other column
            # ranges survives), so the reciprocal batches to a single op.
            # Then batched elu(+1) and log_softmax over the 64 features, with
            # the final subtract + store split in halves across both DMA
            # rings so the tail overlaps. ---
            o_big = opsum.tile([P, RC, EXT], FP32, tag="obig")
            for q in range(RC):
                nc.tensor.transpose(
                    o_big[:, q], outT_sb[:, q * P : (q + 1) * P], ident[:EXT, :EXT]
                )
            rd = work.tile([P, RC, H], FP32, tag="rd")
            nc.vector.reciprocal(
                rd,
                bass.AP(
                    tensor=o_big[:, 0, :].tensor,
                    offset=o_big[:, 0, :].offset + HF,
                    ap=[o_big[:, 0, :].ap[0], [EXT, RC], [1, H]],
                ),
            )
            xo = singles.tile([P, RC, HF], FP32)
            for q in range(RC):
                nc.vector.tensor_mul(
                    xo[:, q].rearrange("p (h f) -> p h f", h=H),
                    o_big[:, q, 0:HF].rearrange("p (h f) -> p h f", h=H),
                    _bcast_f(rd[:, q], F),
                )
            mo = work.tile([P, RC, HF], FP32, tag="mo")
            eo = work.tile([P, RC, HF], FP32, tag="eo")
            yo = singles.tile([P, RC, HF], FP32)
            ex = work.tile([P, RC, HF], FP32, tag="ex")
            sm = work.tile([P, RC], FP32, tag="sm")
            ls = work.tile([P, RC], FP32, tag="ls")
            out_sb = singles.tile([P, RC, HF], FP32)
            ls_base = ls[:]
            HC = RC // 2
            out_view = out[:].rearrange("p (q f) -> p q f", q=RC)
            for half, eng in ((0, nc.sync), (1, nc.scalar)):
                sl = slice(half * HC, (half + 1) * HC)
                flat = lambda t: t[:, sl].rearrange("p q f -> p (q f)")
                # exp(min(x,0)) == min(exp(x),1) exactly (monotonicity):
                # the ACT exp starts straight from xo and the two DVE ops
                # run back-to-back — one cross-engine handoff fewer
                nc.scalar.activation(flat(mo), flat(xo), AFT.Exp)
                nc.vector.tensor_scalar_min(flat(eo), flat(mo), 1.0)
                nc.vector.scalar_tensor_tensor(
                    out=flat(yo), in0=flat(xo), scalar=0.0, in1=flat(eo),
                    op0=ALU.max, op1=ALU.add,
                )
                nc.scalar.activation(flat(ex), flat(yo), AFT.Exp)
                nc.vector.reduce_sum(
                    sm[:, sl], ex[:, sl], axis=mybir.AxisListType.X
                )
                nc.scalar.activation(ls[:, sl], sm[:, sl], AFT.Ln)
                ls_bcast = bass.AP(
                    tensor=ls_base.tensor,
                    offset=ls_base.offset + half * HC,
                    ap=[ls_base.ap[0], [1, HC], [0, HF]],
                )
                nc.vector.tensor_sub(out_sb[:, sl], yo[:, sl], ls_bcast)
                eng.dma_start(out=out_view[:, sl], in_=out_sb[:, sl])

    # Pin all ACT activations (Exp + Ln) onto the single table set holding
    # both so only one ACT_TABLE_LOAD is emitted.
    orig_gat = bacc.get_activation_tables

    def _one_set(arch):
        return {
            k: (v if k == "natural_log_exp_and_others" else set())
            for k, v in orig_gat(arch).items()
        }

    bacc.get_activation_tables = _one_set
    try:
        nc.finalize()
    finally:
        bacc.get_activation_tables = orig_gat
    return nc


def _host_prep(x, adj, W, a_dst, n_nodes):
    """Build per-core input maps (fp8 DoubleRow layouts)."""
    R = n_nodes // N_CORES
    NG = n_nodes // R
    NBLK = n_nodes // 256
    f8 = ml_dtypes.float8_e4m3
    Wd = np.einsum(
        "khf,hf->kh", W.reshape(K_IN, H, F), a_dst, dtype=np.float32
    ).astype(np.float32)
    w_ext = np.zeros((K_IN, EP), dtype=np.float32)
    w_ext[:, :HF] = W * 16.0
    w_ext[:, HF:EXT] = Wd * 64.0
    # wt[p, kk, s, e] = w_ext[kk*256+s*128+p, e]
    wt = np.ascontiguousarray(
        w_ext.reshape(KK, 2, P, EP).transpose(2, 0, 1, 3).reshape(P, KK * 2 * EP)
    ).astype(f8)
    # xt[p, g, kk, s, r] = x[g*512 + r, kk*256 + s*128 + p]  (full x, shared)
    x_f8 = x.astype(f8)
    xt = np.ascontiguousarray(
        x_f8.reshape(NG, R, KK, 2, P)
        .transpose(4, 0, 2, 3, 1)
        .reshape(P, NG * KK * 2 * R)
    )
    adj_f8 = adj.astype(np.int8).astype(f8)  # exact for 0/1
    in_maps = []
    for c in range(N_CORES):
        rows = slice(c * R, (c + 1) * R)
        # at[p, blk, s, r] = adj[c*R + r, blk*256 + s*128 + p]
        ac = adj_f8[rows]  # [512, 4096]
        at = np.ascontiguousarray(
            ac.reshape(R, NBLK, 2, P).transpose(3, 1, 2, 0).reshape(P, NBLK * 2 * R)
        )
        in_maps.append({"xt": xt, "at": at, "wt": wt})
    return in_maps


_BUILT = {}


def run(x, adj, W, a_dst, trace=False):
    n_nodes = x.shape[0]
    R = n_nodes // N_CORES
    RC = R // P
    if n_nodes not in _BUILT:
        _BUILT[n_nodes] = build_bass(n_nodes)
    nc = _BUILT[n_nodes]
    in_maps = _host_prep(x, adj, W, a_dst, n_nodes)
    res = run_bass_kernel_spmd(nc, in_maps, list(range(N_CORES)), trace=trace)
    blocks = []
    for c in range(N_CORES):
        o = res.results[c]["out"]  # [P, RC*HF] p-major
        blocks.append(o.reshape(P, RC, HF).transpose(1, 0, 2).reshape(R, HF))
    return np.concatenate(blocks, axis=0).astype(np.float32), res


def kernel(x, adj, W, a_src, a_dst):
    x = np.asarray(x, dtype=np.float32)
    adj = np.asarray(adj)
    W = np.asarray(W, dtype=np.float32)
    a_dst = np.asarray(a_dst, dtype=np.float32)
    out, _ = run(x, adj, W, a_dst, trace=False)
    return out


# revision 51
# speedup vs baseline: 1.0314x; 1.0314x over previous
"""GAT-style GNN message passing on 8 TRN2 NeuronCores.

Math: with LEAK=1 the leaky-relu is identity, so softmax over j cancels
e_src exactly:
  out[i,(h,f)] = (adj @ (z*h))[i,(h,f)] / (adj @ z)[i,h],  z = exp(h @ a_dst)
then elu + log_softmax per row (elu(x)+1 = relu(x)+exp(min(x,0)); the +1
is a uniform shift which log_softmax cancels).

Parallelisation: rows (query nodes) of adj/out are sharded across the 8
cores, but x is REPLICATED and every core computes the full [N, 72]
G = [h*z | z] locally. The AllGather alternative (gather 40KB of G per
core) measures ~46us end-to-end on this runtime (ncfw trigger->mesh
latency dominates) and couples the cores' launch skew; replicating the
x read costs only ~10us of extra DMA and makes each core's execution
completely independent.

All matmuls run as fp8e4 DoubleRow (2 contractions/cycle):
 - adj entries are 0/1 (exact in fp8); per-core slab is 2MB.
 - x, W are fp8 with power-of-2 column scaling so values sit in e4m3's
   normal range: h' = x @ (16W) = 16h, e' = x @ (64Wd) = 64*e_dst.
   z = exp(e'/64) (ACT scale), g = (h'/16)*z via one scalar_tensor_tensor
   that reuses the rounded fp8 z (numerator and denominator then share
   the same z, so z rounding only reweights attention by ~0.06/sqrt(2048)).
 - dominant error is g/h rounding ~10% of the signal; |out| ~ 0.04 vs
   log_softmax scale ~4.6 => rel err ~ 2e-3, well under the 2e-2 gate.
 - E is padded 72->80 so the DoubleRow pair stride (80B) is 16B-aligned
   (walrus double_row_stride_alignment). Pad columns only produce junk
   PSUM rows 72:80 which are never read (zeroed once for determinism).

The 2MB adjacency streams behind a real data dependency on the x load
(Tile hoists dependency-free DMAs, so the gate is one garbage byte
copied from xt_sb into each chunk's region, overwritten by the DMA);
it is only needed by the aggregation ~10us after x completes.

Per-core layouts (R=512 out-rows/core, P=128, KK=4 k-blocks of 256,
G=8 node column-groups of 512):
  xt [128, 8*4*2*512] fp8  xt[p, g,kk,s,r] = x[g*512 + r, kk*256+s*128+p]
                           (full x, same buffer on every core)
  wt [128, 4*2*80]    fp8  wt[p, kk,s,e]   = w_ext[kk*256+s*128+p, e]
  at [128, 16*2*512]  fp8  at[p, blk,s,r]  = adj[c*R + r, blk*256+s*128+p]
  out [128, 4*64]     f32  out[p, q*64+f]  = out[c*R + q*128 + p, f]
"""

import sys

import numpy as np

if "/opt/trn_rl_repo" not in sys.path:
    sys.path.insert(0, "/opt/trn_rl_repo")

import ml_dtypes  # noqa: E402

import concourse.bass as bass  # noqa: E402
import concourse.tile as tile  # noqa: E402
from concourse import bacc, mybir  # noqa: E402
from concourse.bass_utils import run_bass_kernel_spmd  # noqa: E402
from concourse.masks import make_identity  # noqa: E402

N_CORES = 8
H = 8
F = 8
HF = H * F  # 64
EXT = HF + H  # 72: [g | z]
EP = 80  # padded to 16B-aligned DoubleRow pair stride
K_IN = 1024
P = 128
KK = K_IN // 256  # 4 double-row k-blocks for the h matmul

FP32 = mybir.dt.float32
FP8 = mybir.dt.float8e4
AFT = mybir.ActivationFunctionType
ALU = mybir.AluOpType
DR = mybir.MatmulPerfMode.DoubleRow


def _bcast_f(ap_ph, n):
    """[P, H] AP -> [P, H, n] AP broadcasting each head value over n."""
    return bass.AP(
        tensor=ap_ph.tensor,
        offset=ap_ph.offset,
        ap=[ap_ph.ap[0], ap_ph.ap[1], [0, n]],
    )


def build_bass(n_nodes: int) -> bass.Bass:
    R = n_nodes // N_CORES  # 512 output rows per core
    NG = n_nodes // R  # 8 node column-groups (of 512) for the h matmul
    NBLK = n_nodes // 256  # 16 double-row j-blocks for the aggregation
    RC = R // P  # 4 output row chunks per core

    nc = bacc.Bacc(num_devices=N_CORES)

    xt = nc.declare_dram_parameter(
        "xt", [P, NG * KK * 2 * (R // 1)], FP8, isOutput=False
    )
    at = nc.declare_dram_parameter("at", [P, NBLK * 2 * R], FP8, isOutput=False)
    wt = nc.declare_dram_parameter("wt", [P, KK * 2 * EP], FP8, isOutput=False)
    out = nc.declare_dram_parameter("out", [P, RC * HF], FP32, isOutput=True)

    with tile.TileContext(nc) as tc:
        with (
            tc.tile_pool(name="singles", bufs=1) as singles,
            tc.tile_pool(name="bigpsum", bufs=2, space="PSUM") as bigpsum,
            tc.tile_pool(name="smallpsum", bufs=4, space="PSUM") as smallpsum,
            tc.tile_pool(name="opsum", bufs=1, space="PSUM") as opsum,
            tc.tile_pool(name="aggpsum", bufs=1, space="PSUM") as aggpsum,
            tc.tile_pool(name="work", bufs=4) as work,
        ):
            ident = singles.tile([P, P], FP32)
            make_identity(nc, ident)
            identb = singles.tile([P, P], mybir.dt.bfloat16)
            make_identity(nc, identb)

            # --- critical-path loads: wt first, then x group by group ---
            w_sb = singles.tile([P, KK, 2, EP], FP8)
            nc.sync.dma_start(
                out=w_sb, in_=wt[:].rearrange("p (k s e) -> p k s e", k=KK, s=2)
            )
            xt_sb = singles.tile([P, NG, KK, 2, R], FP8)
            xt_view = xt[:].rearrange(
                "p (g k s r) -> p g k s r", g=NG, k=KK, s=2
            )
            # first two groups load alone so the PE starts ~2us earlier
            xt_groups = [(0, 1), (1, 2), (2, 4), (4, 6), (6, 8)]
            for lo, hi in xt_groups:
                nc.sync.dma_start(out=xt_sb[:, lo:hi], in_=xt_view[:, lo:hi])

            # --- adjacency gated behind the x load (SWDGE ring) ---
            at_sb = singles.tile([P, NBLK, 2, R], FP8)
            at_view = at[:].rearrange("p (b s r) -> p b s r", b=NBLK, s=2)
            N_SPLITS = 4
            # gate on xt group 2: early enough that the adjacency's last
            # chunk lands well before the aggregation (a later gate stalls
            # the agg matmuls: measured +6us at group 6), at the cost of
            # some bandwidth-sharing with the xt tail
            for sp in range(N_SPLITS):
                lo = NBLK // N_SPLITS * sp
                nc.vector.tensor_copy(
                    at_sb[0:1, lo, 0, 0:1],
                    xt_sb[0:1, 2, 0, 0, sp : sp + 1],
                )
            for sp in range(N_SPLITS):
                lo, hi = NBLK // N_SPLITS * sp, NBLK // N_SPLITS * (sp + 1)
                nc.gpsimd.dma_start(out=at_sb[:, lo:hi], in_=at_view[:, lo:hi])

            # --- full G production: per column-group of 512 nodes,
            # h' = (16W|64Wd).T @ x.T, transpose 128-chunks,
            # z = exp(e'/64), g = (h'/16)*z ---
            g2 = singles.tile([P, NBLK, 2, EP], FP8)
            g2_base = g2[:, 0, 0, :]
            pad_ap = bass.AP(
                tensor=g2_base.tensor,
                offset=g2_base.offset + EXT,
                ap=[g2_base.ap[0], [EP, NBLK * 2], [1, EP - EXT]],
            )
            nc.vector.memset(pad_ap, 0.0)

            # bf16 staging: halves the PSUM->SBUF copy and the transpose
            # traffic; h'/e' at 0.4% rel err is negligible vs fp8's 6%
            BF16 = mybir.dt.bfloat16
            hT_sb = singles.tile([EXT, NG, R], BF16)
            outT_ps = aggpsum.tile([EP, R], FP32, tag="aggps")
            for g in range(NG):
                hT_ps = bigpsum.tile([EP, R], FP32, tag="bigps", name=f"hT{g}")
                for k in range(KK):
                    nc.tensor.matmul(
                        hT_ps,
                        lhsT=w_sb[:, k],
                        rhs=xt_sb[:, g, k],
                        start=(k == 0),
                        stop=(k == KK - 1),
                        perf_mode=DR,
                    )
                # staging copy on ACT (reads PSUM fast, otherwise idle here);
                # frees DVE for the scalar_tensor_tensor ladder
                nc.scalar.activation(hT_sb[:, g], hT_ps[:EXT], AFT.Copy)
                for qq in range(RC):
                    q = g * RC + qq
                    h_ps = smallpsum.tile([P, EXT], mybir.dt.bfloat16, tag="smallps")
                    nc.tensor.transpose(
                        h_ps,
                        hT_sb[:, g, qq * P : (qq + 1) * P],
                        identb[:EXT, :EXT],
                    )
                    zslice = g2[:, q // 2, q % 2, HF:EXT]
                    nc.scalar.activation(
                        zslice, h_ps[:, HF:EXT], AFT.Exp, scale=1.0 / 64.0
                    )
                    nc.vector.scalar_tensor_tensor(
                        out=g2[:, q // 2, q % 2, 0:HF].rearrange(
                            "p (h f) -> p h f", h=H
                        ),
                        in0=h_ps[:, 0:HF].rearrange("p (h f) -> p h f", h=H),
                        scalar=1.0 / 16.0,
                        in1=_bcast_f(zslice, F),
                        op0=ALU.mult,
                        op1=ALU.mult,
                    )
            # --- aggregation: outT += G_blk.T @ adjT_blk (16 DR matmuls;
            # kept after the group loop — interleaving them stalls the FIFO
            # PE queue on adjacency chunks that are still streaming) ---
            for blk in range(NBLK):
                nc.tensor.matmul(
                    outT_ps,
                    lhsT=g2[:, blk],
                    rhs=at_sb[:, blk],
                    start=(blk == 0),
                    stop=(blk == NBLK - 1),
                    perf_mode=DR,
                )
            outT_sb = singles.tile([EXT, R], FP32)
            nc.vector.tensor_copy(outT_sb, outT_ps[:EXT])

            # --- postprocess: all 4 transposed chunks land in ONE PSUM bank
            # (start=True only clears has_written bits, data in other column
            # ranges survives), so the reciprocal batches to a single op.
            # Then batched elu(+1) and log_softmax over the 64 features, with
            # the final subtract + store split in halves across both DMA
            # rings so the tail overlaps. ---
            o_big = opsum.tile([P, RC, EXT], FP32, tag="obig")
            for q in range(RC):
                nc.tensor.transpose(
                    o_big[:, q], outT_sb[:, q * P : (q + 1) * P], ident[:EXT, :EXT]
                )
            rd = work.tile([P, RC, H], FP32, tag="rd")
            nc.vector.reciprocal(
                rd,
                bass.AP(
                    tensor=o_big[:, 0, :].tensor,
                    offset=o_big[:, 0, :].offset + HF,
                    ap=[o_big[:, 0, :].ap[0], [EXT, RC], [1, H]],
                ),
            )
            xo = singles.tile([P, RC, HF], FP32)
            for q in range(RC):
                nc.vector.tensor_mul(
                    xo[:, q].rearrange("p (h f) -> p h f", h=H),
                    o_big[:, q, 0:HF].rearrange("p (h f) -> p h f", h=H),
                    _bcast_f(rd[:, q], F),
                )
            mo = work.tile([P, RC, HF], FP32, tag="mo")
            eo = work.tile([P, RC, HF], FP32, tag="eo")
            yo = singles.tile([P, RC, HF], FP32)
            ex = work.tile([P, RC, HF], FP32, tag="ex")
            sm = work.tile([P, RC], FP32, tag="sm")
            ls = work.tile([P, RC], FP32, tag="ls")
            out_sb = singles.tile([P, RC, HF], FP32)
            ls_base = ls[:]
            HC = RC // 2
            out_view = out[:].rearrange("p (q f) -> p q f", q=RC)
            for half, eng in ((0, nc.sync), (1, nc.scalar)):
                sl = slice(half * HC, (half + 1) * HC)
                flat = lambda t: t[:, sl].rearrange("p q f -> p (q f)")
                nc.vector.tensor_scalar_min(flat(mo), flat(xo), 0.0)
                nc.scalar.activation(flat(eo), flat(mo), AFT.Exp)
                nc.vector.scalar_tensor_tensor(
                    out=flat(yo), in0=flat(xo), scalar=0.0, in1=flat(eo),
                    op0=ALU.max, op1=ALU.add,
                )
                nc.scalar.activation(flat(ex), flat(yo), AFT.Exp)
                nc.vector.reduce_sum(
                    sm[:, sl], ex[:, sl], axis=mybir.AxisListType.X
                )
                nc.scalar.activation(ls[:, sl], sm[:, sl], AFT.Ln)
                ls_bcast = bass.AP(
                    tensor=ls_base.tensor,
                    offset=ls_base.offset + half * HC,
                    ap=[ls_base.ap[0], [1, HC], [0, HF]],
                )
                nc.vector.tensor_sub(out_sb[:, sl], yo[:, sl], ls_bcast)
                eng.dma_start(out=out_view[:, sl], in_=out_sb[:, sl])

    # Pin all ACT activations (Exp + Ln) onto the single table set holding
    # both so only one ACT_TABLE_LOAD is emitted.
    orig_gat = bacc.get_activation_tables

    def _one_set(arch):
        return {
            k: (v if k == "natural_log_exp_and_others" else set())
            for k, v in orig_gat(arch).items()
        }

    bacc.get_activation_tables = _one_set
    try:
        nc.finalize()
    finally:
        bacc.get_activation_tables = orig_gat
    return nc


def _host_prep(x, adj, W, a_dst, n_nodes):
    """Build per-core input maps (fp8 DoubleRow layouts)."""
    R = n_nodes // N_CORES
    NG = n_nodes // R
    NBLK = n_nodes // 256
    f8 = ml_dtypes.float8_e4m3
    Wd = np.einsum(
        "khf,hf->kh", W.reshape(K_IN, H, F), a_dst, dtype=np.float32
    ).astype(np.float32)
    w_ext = np.zeros((K_IN, EP), dtype=np.float32)
    w_ext[:, :HF] = W * 16.0
    w_ext[:, HF:EXT] = Wd * 64.0
    # wt[p, kk, s, e] = w_ext[kk*256+s*128+p, e]
    wt = np.ascontiguousarray(
        w_ext.reshape(KK, 2, P, EP).transpose(2, 0, 1, 3).reshape(P, KK * 2 * EP)
    ).astype(f8)
    # xt[p, g, kk, s, r] = x[g*512 + r, kk*256 + s*128 + p]  (full x, shared)
    x_f8 = x.astype(f8)
    xt = np.ascontiguousarray(
        x_f8.reshape(NG, R, KK, 2, P)
        .transpose(4, 0, 2, 3, 1)
        .reshape(P, NG * KK * 2 * R)
    )
    adj_f8 = adj.astype(np.int8).astype(f8)  # exact for 0/1
    in_maps = []
    for c in range(N_CORES):
        rows = slice(c * R, (c + 1) * R)
        # at[p, blk, s, r] = adj[c*R + r, blk*256 + s*128 + p]
        ac = adj_f8[rows]  # [512, 4096]
        at = np.ascontiguousarray(
            ac.reshape(R, NBLK, 2, P).transpose(3, 1, 2, 0).reshape(P, NBLK * 2 * R)
        )
        in_maps.append({"xt": xt, "at": at, "wt": wt})
    return in_maps


_BUILT = {}


def run(x, adj, W, a_dst, trace=False):
    n_nodes = x.shape[0]
    R = n_nodes // N_CORES
    RC = R // P
    if n_nodes not in _BUILT:
        _BUILT[n_nodes] = build_bass(n_nodes)
    nc = _BUILT[n_nodes]
    in_maps = _host_prep(x, adj, W, a_dst, n_nodes)
    res = run_bass_kernel_spmd(nc, in_maps, list(range(N_CORES)), trace=trace)
    blocks = []
    for c in range(N_CORES):
        o = res.results[c]["out"]  # [P, RC*HF] p-major
        blocks.append(o.reshape(P, RC, HF).transpose(1, 0, 2).reshape(R, HF))
    return np.concatenate(blocks, axis=0).astype(np.float32), res


def kernel(x, adj, W, a_src, a_dst):
    x = np.asarray(x, dtype=np.float32)
    adj = np.asarray(adj)
    W = np.asarray(W, dtype=np.float32)
    a_dst = np.asarray(a_dst, dtype=np.float32)
    out, _ = run(x, adj, W, a_dst, trace=False)
    return out
